# revision 6
# baseline (speedup 1.0000x reference)
"""Multi-head attention (B=2, S=2048, D=1024, H=16) on 8 TRN2 NeuronCores.

Sharding: core c handles batch b = c//4 and head group hg = c%4 (4 heads,
256 features f0 = hg*256). Each core computes Q/K/V projections for its
feature slice, attention for its 4 heads, and a partial output projection
y_partial = attnout @ Wo[:, f0:f0+256].T. Host sums the 4 partials per batch
and adds bo.

On-device layout strategy (everything contraction-dim-on-partitions):
 - host pre-transposes x -> xT [D, S] and weight slices -> [D, 256] so no
   on-device transposes are needed.
 - Q, K are produced transposed: QT/KT [e, s] (e = 256 local features).
 - scores are produced transposed per head: ST [k, q] = K_h @ Q_h.T, packed
   two heads at a time in the PE array (row groups 0-63 / 64-127, K=64 each).
 - softmax: exp(0.125 * ST) with no max subtraction (scores are ~N(0,1) by
   construction so exp is safe in fp32); row sums come from an extra
   ones-column in the AV matmul; normalization multiplies by 1/sum broadcast
   across partitions via a K=1 ones-matmul.
 - AV: OT_h [hd, q] = V_aug_h.T @ Pexp (contraction over k, full K=128).
   Even heads put the ones column at index 64 (sums at out partition 64),
   odd heads use a padded 128-wide weight with ones at col 0 and V at cols
   64..127 so their hd lands on out partitions 64..127; the normalized
   outputs of an even/odd pair form a full 128-partition attnout.T chunk.
 - out-proj: y[s-tile] = sum_cc OT[:, cc, s-tile].T @ woT[cc] (K=128 x2).

All matmuls run in float32r (fp32 rounded to E8M11), which streams at full
PE rate for free dims >= 256. Inputs are pre-rounded on the host; on-device
matmul operands are produced by DVE/ACT ops writing float32r tiles.
"""
import numpy as np

import concourse.bass as bass
import concourse.mybir as mybir
import concourse.tile as tile
from concourse import bacc
from concourse import bass_utils

F32 = mybir.dt.float32
F32R = mybir.dt.float32r
EXP = mybir.ActivationFunctionType.Exp
ADD = mybir.AluOpType.add
MULT = mybir.AluOpType.mult

B, S, D, H = 2, 2048, 1024, 16
HD = D // H          # 64
E = 256              # local features per core (4 heads)
QC = 256             # q-chunk size for the attention phase
N_QC = S // QC       # 8
N_KT = S // 128      # 16 k-tiles
N_ST = S // 128      # 16 s-tiles
KD = D // 128        # 8 contraction tiles for the projections


def round_fp32r(a: np.ndarray) -> np.ndarray:
    """Round fp32 to E8M11 (float32r): RNE at mantissa bit 12."""
    u = np.ascontiguousarray(a, dtype=np.float32).view(np.uint32).copy()
    lsb = (u >> 12) & 1
    u = (u + 0x7FF + lsb) & np.uint32(0xFFFFF000)
    return u.view(np.float32)


def build():
    nc = bacc.Bacc("TRN2", target_bir_lowering=False, debug=False, num_devices=8)

    xT = nc.dram_tensor("xT", [D, S], F32R, kind="ExternalInput").ap()
    wqT = nc.dram_tensor("wqT", [D, E], F32R, kind="ExternalInput").ap()
    wkT = nc.dram_tensor("wkT", [D, E], F32R, kind="ExternalInput").ap()
    wvT = nc.dram_tensor("wvT", [D, E], F32R, kind="ExternalInput").ap()
    woT = nc.dram_tensor("woT", [E, D], F32R, kind="ExternalInput").ap()
    bq2 = nc.dram_tensor("bq2", [128, 2], F32, kind="ExternalInput").ap()
    bk2 = nc.dram_tensor("bk2", [128, 2], F32, kind="ExternalInput").ap()
    bvb = nc.dram_tensor("bvb", [128, E], F32, kind="ExternalInput").ap()
    # [:, :, 0] = 1.0, rest 0 -- supplies the ones/zeros columns of V_aug
    vcon = nc.dram_tensor("vcon", [128, N_KT, 64], F32R, kind="ExternalInput").ap()
    ones64 = nc.dram_tensor("ones64", [1, 128], F32R, kind="ExternalInput").ap()

    y = nc.dram_tensor("y", [S, D], F32, kind="ExternalOutput").ap()

    with tile.TileContext(nc) as tc:
        with (
            tc.tile_pool(name="persist", bufs=1) as pp,
            tc.tile_pool(name="ps_proj", bufs=2, space="PSUM") as ps_proj,
            tc.tile_pool(name="ps_s", bufs=3, space="PSUM") as ps_s,
            tc.tile_pool(name="ps_av", bufs=3, space="PSUM") as ps_av,
        ):
            # ---------------- persistent tiles ----------------
            woT_sb = pp.tile([128, 2, D], F32R)
            bvb_sb = pp.tile([128, E], F32)
            vcon_sb = pp.tile([128, N_KT, 64], F32R)
            ones_sb = pp.tile([128, 128], F32R)    # partitions 0 and 64 used
            bq_sb = pp.tile([128, 2], F32)
            bk_sb = pp.tile([128, 2], F32)
            QT_sb = pp.tile([128, 2, S], F32R)
            KT_sb = pp.tile([128, 2, S], F32R)
            OT_sb = pp.tile([128, 2, S], F32R)
            # V_aug per pair (128 cols each so the matmul dst is a full
            # 128-partition AP):
            #   even head: [*, kt, 0:64]=V, col 64=1, cols 65:128=0
            #   odd head:  col 0=1, cols 1:64=0, [*, kt, 64:128]=V
            Ve_sb = [pp.tile([128, N_KT, 128], F32R, name=f"ve{p}", tag=f"ve{p}")
                     for p in range(2)]
            Vo_sb = [pp.tile([128, N_KT, 128], F32R, name=f"vo{p}", tag=f"vo{p}")
                     for p in range(2)]

            for p in range(2):
                nc.sync.dma_start(woT_sb[:, p, :], woT[p * 128:(p + 1) * 128, :])
            nc.sync.dma_start(bvb_sb[:], bvb)
            nc.sync.dma_start(vcon_sb[:], vcon)
            nc.sync.dma_start(ones_sb[0:1, :], ones64)
            nc.sync.dma_start(ones_sb[64:65, :], ones64)
            nc.sync.dma_start(bq_sb[:], bq2)
            nc.sync.dma_start(bk_sb[:], bk2)
            # ---------------- phase 1: projections ----------------
            with tc.tile_pool(name="ph1", bufs=1) as p1:
                xT_sb = p1.tile([128, KD, S], F32R)
                wq_sb = p1.tile([128, KD, E], F32R)
                wk_sb = p1.tile([128, KD, E], F32R)
                wv_sb = p1.tile([128, KD, E], F32R)
                for k in range(KD):
                    nc.sync.dma_start(wq_sb[:, k, :], wqT[k * 128:(k + 1) * 128, :])
                    nc.sync.dma_start(wk_sb[:, k, :], wkT[k * 128:(k + 1) * 128, :])
                    nc.sync.dma_start(wv_sb[:, k, :], wvT[k * 128:(k + 1) * 128, :])
                for k in range(KD):
                    nc.sync.dma_start(xT_sb[:, k, :], xT[k * 128:(k + 1) * 128, :])

                # QT / KT: [e-chunk(128), s] = W.T @ xT
                for w_sb, b_sb, out_sb in ((wq_sb, bq_sb, QT_sb),
                                           (wk_sb, bk_sb, KT_sb)):
                    for ec in range(2):
                        for sc in range(S // 512):
                            ps = ps_proj.tile([128, 512], F32, tag="proj")
                            for k in range(KD):
                                nc.tensor.matmul(
                                    ps[:],
                                    w_sb[:, k, ec * 128:(ec + 1) * 128],
                                    xT_sb[:, k, sc * 512:(sc + 1) * 512],
                                    start=(k == 0), stop=(k == KD - 1))
                            nc.vector.tensor_scalar(
                                out_sb[:, ec, sc * 512:(sc + 1) * 512], ps[:],
                                b_sb[:, ec:ec + 1], None, ADD)

                # V: [s-tile(128), e(256)] = xT.T @ wvT, scattered into V_aug
                for st in range(N_ST):
                    ps = ps_proj.tile([128, 512], F32, tag="proj")
                    for k in range(KD):
                        nc.tensor.matmul(
                            ps[:, 0:E],
                            xT_sb[:, k, st * 128:(st + 1) * 128],
                            wv_sb[:, k, :],
                            start=(k == 0), stop=(k == KD - 1))
                    for h in range(4):
                        pr, odd = h // 2, h % 2
                        dst = (Vo_sb[pr][:, st, 64:128] if odd
                               else Ve_sb[pr][:, st, 0:64])
                        nc.vector.tensor_tensor(
                            dst, ps[:, h * 64:(h + 1) * 64],
                            bvb_sb[:, h * 64:(h + 1) * 64], ADD)

            # ones/zeros columns of V_aug (SBUF->SBUF DMA from vcon_sb)
            for pr in range(2):
                nc.sync.dma_start(Ve_sb[pr][:, :, 64:128], vcon_sb[:, :, :])
                nc.sync.dma_start(Vo_sb[pr][:, :, 0:64], vcon_sb[:, :, :])

            # ---------------- phase 2+3: attention + out-proj ----------------
            with tc.tile_pool(name="ph2", bufs=4) as p2:
                for qc in range(N_QC):
                    for pr in range(2):
                        pexp_e = p2.tile([128, N_KT, QC], F32R, tag="pexp")
                        pexp_o = p2.tile([128, N_KT, QC], F32R, tag="pexp")
                        qsl = slice(qc * QC, (qc + 1) * QC)
                        for kt in range(N_KT):
                            ksl = slice(kt * 128, (kt + 1) * 128)
                            pse = ps_s.tile([128, QC], F32, tag="s")
                            pso = ps_s.tile([128, QC], F32, tag="s")
                            nc.tensor.matmul(pse[:], KT_sb[0:64, pr, ksl],
                                             QT_sb[0:64, pr, qsl])
                            nc.tensor.matmul(pso[:], KT_sb[64:128, pr, ksl],
                                             QT_sb[64:128, pr, qsl])
                            nc.scalar.activation(pexp_e[:, kt, :], pse[:],
                                                 EXP, scale=0.125)
                            nc.scalar.activation(pexp_o[:, kt, :], pso[:],
                                                 EXP, scale=0.125)

                        for odd, pexp in ((0, pexp_e), (1, pexp_o)):
                            v_sb = Vo_sb[pr] if odd else Ve_sb[pr]
                            psav = ps_av.tile([128, QC], F32, tag="av")
                            for kt in range(N_KT):
                                nc.tensor.matmul(
                                    psav[:], v_sb[:, kt, :],
                                    pexp[:, kt, :],
                                    start=(kt == 0), stop=(kt == N_KT - 1))
                            # normalization: sums at partition 64 (even) / 0 (odd)
                            sp = 0 if odd else 64      # sums partition
                            op = 64 if odd else 0      # hd base out partition
                            rec = p2.tile([128, QC], F32, tag="rec")
                            recr = p2.tile([128, QC], F32R, tag="recr")
                            nc.vector.reciprocal(rec[sp:sp + 1, :],
                                                 psav[sp:sp + 1, :])
                            nc.vector.tensor_copy(recr[sp:sp + 1, :],
                                                  rec[sp:sp + 1, :])
                            psbc = ps_av.tile([128, QC], F32, tag="av")
                            nc.tensor.matmul(psbc[:],
                                             ones_sb[sp:sp + 1, :],
                                             recr[sp:sp + 1, :])
                            bcs = p2.tile([128, QC], F32, tag="bcs")
                            nc.vector.tensor_copy(bcs[op:op + 64, :],
                                                  psbc[op:op + 64, :])
                            nc.vector.tensor_tensor(
                                OT_sb[op:op + 64, pr, qsl],
                                psav[op:op + 64, :] if odd
                                else psav[0:64, :],
                                bcs[op:op + 64, :], MULT)

                    # out-proj for the two s-tiles covered by this q-chunk
                    for sti in range(QC // 128):
                        st = qc * (QC // 128) + sti
                        ssl = slice(st * 128, (st + 1) * 128)
                        for nch in range(2):
                            psy = ps_proj.tile([128, 512], F32, tag="proj")
                            for cc in range(2):
                                nc.tensor.matmul(
                                    psy[:], OT_sb[:, cc, ssl],
                                    woT_sb[:, cc, nch * 512:(nch + 1) * 512],
                                    start=(cc == 0), stop=(cc == 1))
                            y_sb = p2.tile([128, 512], F32, tag="y")
                            nc.vector.tensor_copy(y_sb[:], psy[:])
                            nc.sync.dma_start(
                                y[ssl, nch * 512:(nch + 1) * 512], y_sb[:])

    nc.compile()
    return nc


_NC_CACHE = None


def kernel(x, Wq, bq, Wk, bk, Wv, bv, Wo, bo):
    global _NC_CACHE
    x = np.asarray(x, dtype=np.float32)
    Wq, bq = np.asarray(Wq, np.float32), np.asarray(bq, np.float32)
    Wk, bk = np.asarray(Wk, np.float32), np.asarray(bk, np.float32)
    Wv, bv = np.asarray(Wv, np.float32), np.asarray(bv, np.float32)
    Wo, bo = np.asarray(Wo, np.float32), np.asarray(bo, np.float32)

    if _NC_CACHE is None:
        _NC_CACHE = build()
    nc = _NC_CACHE

    vcon = np.zeros((128, N_KT, 64), np.float32)
    vcon[:, :, 0] = 1.0
    ones64 = np.ones((1, 128), np.float32)

    in_maps = []
    for c in range(8):
        b, f0 = c // 4, (c % 4) * E
        fs = slice(f0, f0 + E)
        in_maps.append(dict(
            xT=round_fp32r(x[b].T),
            wqT=round_fp32r(Wq[fs, :].T),
            wkT=round_fp32r(Wk[fs, :].T),
            wvT=round_fp32r(Wv[fs, :].T),
            woT=round_fp32r(Wo[:, fs].T),
            bq2=np.ascontiguousarray(bq[fs].reshape(2, 128).T),
            bk2=np.ascontiguousarray(bk[fs].reshape(2, 128).T),
            bvb=np.ascontiguousarray(np.broadcast_to(bv[fs], (128, E))),
            vcon=vcon,
            ones64=ones64,
        ))

    global last_in_maps
    last_in_maps = in_maps
    res = bass_utils.run_bass_kernel_spmd(nc, in_maps, core_ids=list(range(8)))

    out = np.zeros((B, S, D), np.float32)
    for c in range(8):
        out[c // 4] += res.results[c]["y"]
    out += bo
    return out


# revision 7
# speedup vs baseline: 1.6992x; 1.6992x over previous
"""Multi-head attention (B=2, S=2048, D=1024, H=16) on 8 TRN2 NeuronCores.

Sharding: core c handles batch b = c//4 and head group hg = c%4 (4 heads,
256 features f0 = hg*256). Each core computes Q/K/V projections for its
feature slice, attention for its 4 heads, and a partial output projection
y_partial = attnout @ Wo[:, f0:f0+256].T. Host sums the 4 partials per batch
and adds bo.

On-device layout strategy (everything contraction-dim-on-partitions):
 - host pre-transposes x -> xT [D, S] and weight slices -> [D, 256] so no
   on-device transposes are needed.
 - Q, K are produced transposed: QT/KT [e, s] (e = 256 local features).
 - scores are produced transposed per head: ST [k, q] = K_h @ Q_h.T, packed
   two heads at a time in the PE array (row groups 0-63 / 64-127, K=64 each).
 - softmax: exp(0.125 * ST) with no max subtraction (scores are ~N(0,1) by
   construction so exp is safe); row sums come from an extra ones column in
   the AV matmul; normalization multiplies by 1/sum broadcast across
   partitions via a K=1 ones-matmul.
 - AV: OT_h [hd, q] = V_aug_h.T @ Pexp (contraction over k, full K=128).
   Even heads: V at cols 0:64, ones at col 64 (sums at out partition 64);
   odd heads: ones at col 0, V at cols 64:128 (hd lands on partitions
   64:128). A normalized even/odd pair forms a full 128-partition
   attnout.T chunk.
 - out-proj: y[s-tile] = sum_cc OT[:, cc, s-tile].T @ woT[cc] (K=128 x2).

All matmuls run in float16 (1 cycle/row at 2.4GHz warm, FWL fast weight
load); accumulation is fp32 in PSUM, elementwise work is fp32 on DVE/ACT.
"""
import numpy as np

import concourse.bass as bass
import concourse.mybir as mybir
import concourse.tile as tile
from concourse import bacc
from concourse import bass_utils

F32 = mybir.dt.float32
F16 = mybir.dt.float16
EXP = mybir.ActivationFunctionType.Exp
ADD = mybir.AluOpType.add
MULT = mybir.AluOpType.mult

B, S, D, H = 2, 2048, 1024, 16
HD = D // H          # 64
E = 256              # local features per core (4 heads)
QC = 512             # q-chunk size for the attention phase
N_QC = S // QC       # 4
N_KT = S // 128      # 16 k-tiles
N_ST = S // 128      # 16 s-tiles
KD = D // 128        # 8 contraction tiles for the projections


def build():
    nc = bacc.Bacc("TRN2", target_bir_lowering=False, debug=False, num_devices=8)

    xT = nc.dram_tensor("xT", [D, S], F16, kind="ExternalInput").ap()
    wqT = nc.dram_tensor("wqT", [D, E], F16, kind="ExternalInput").ap()
    wkT = nc.dram_tensor("wkT", [D, E], F16, kind="ExternalInput").ap()
    wvT = nc.dram_tensor("wvT", [D, E], F16, kind="ExternalInput").ap()
    woT = nc.dram_tensor("woT", [E, D], F16, kind="ExternalInput").ap()
    bq2 = nc.dram_tensor("bq2", [128, 2], F32, kind="ExternalInput").ap()
    bk2 = nc.dram_tensor("bk2", [128, 2], F32, kind="ExternalInput").ap()
    bvb = nc.dram_tensor("bvb", [128, E], F32, kind="ExternalInput").ap()
    # [:, :, 0] = 1.0, rest 0 -- supplies the ones/zeros columns of V_aug
    vcon = nc.dram_tensor("vcon", [128, N_KT, 64], F16, kind="ExternalInput").ap()
    ones128 = nc.dram_tensor("ones128", [1, 128], F16, kind="ExternalInput").ap()

    y = nc.dram_tensor("y", [S, D], F32, kind="ExternalOutput").ap()

    with tile.TileContext(nc) as tc:
        with (
            tc.tile_pool(name="pool", bufs=1) as pp,
            tc.tile_pool(name="work", bufs=4) as wk,
            tc.tile_pool(name="ps_proj", bufs=2, space="PSUM") as ps_proj,
            tc.tile_pool(name="ps_s", bufs=3, space="PSUM") as ps_s,
            tc.tile_pool(name="ps_av", bufs=3, space="PSUM") as ps_av,
        ):
            # ---------------- persistent tiles ----------------
            woT_sb = pp.tile([128, 2, D], F16)
            bvb_sb = pp.tile([128, E], F32)
            vcon_sb = pp.tile([128, N_KT, 64], F16)
            ones_sb = pp.tile([128, 128], F16)     # partitions 0 and 64 used
            bq_sb = pp.tile([128, 2], F32)
            bk_sb = pp.tile([128, 2], F32)
            QT_sb = pp.tile([128, 2, S], F16)
            KT_sb = pp.tile([128, 2, S], F16)
            OT_sb = pp.tile([128, 2, S], F16)
            xT_sb = pp.tile([128, KD, S], F16)
            wq_sb = pp.tile([128, KD, E], F16)
            wk_sb = pp.tile([128, KD, E], F16)
            wv_sb = pp.tile([128, KD, E], F16)
            # V_aug per pair (128 cols each so the matmul dst is a full
            # 128-partition AP):
            #   even head: [*, kt, 0:64]=V, col 64=1, cols 65:128=0
            #   odd head:  col 0=1, cols 1:64=0, [*, kt, 64:128]=V
            Ve_sb = [pp.tile([128, N_KT, 128], F16, name=f"ve{p}", tag=f"ve{p}")
                     for p in range(2)]
            Vo_sb = [pp.tile([128, N_KT, 128], F16, name=f"vo{p}", tag=f"vo{p}")
                     for p in range(2)]

            for p in range(2):
                nc.sync.dma_start(woT_sb[:, p, :], woT[p * 128:(p + 1) * 128, :])
            nc.sync.dma_start(bvb_sb[:], bvb)
            nc.sync.dma_start(vcon_sb[:], vcon)
            nc.sync.dma_start(ones_sb[0:1, :], ones128)
            nc.sync.dma_start(ones_sb[64:65, :], ones128)
            nc.sync.dma_start(bq_sb[:], bq2)
            nc.sync.dma_start(bk_sb[:], bk2)
            for k in range(KD):
                nc.sync.dma_start(wq_sb[:, k, :], wqT[k * 128:(k + 1) * 128, :])
                nc.sync.dma_start(wk_sb[:, k, :], wkT[k * 128:(k + 1) * 128, :])
                nc.sync.dma_start(wv_sb[:, k, :], wvT[k * 128:(k + 1) * 128, :])
            for k in range(KD):
                nc.sync.dma_start(xT_sb[:, k, :], xT[k * 128:(k + 1) * 128, :])
            for pr in range(2):
                nc.sync.dma_start(Ve_sb[pr][:, :, 64:128], vcon_sb[:, :, :])
                nc.sync.dma_start(Vo_sb[pr][:, :, 0:64], vcon_sb[:, :, :])

            # ---------------- phase 1: projections ----------------
            # QT / KT: [e-chunk(128), s] = W.T @ xT
            for w_sb, b_sb, out_sb in ((wq_sb, bq_sb, QT_sb),
                                       (wk_sb, bk_sb, KT_sb)):
                for ec in range(2):
                    for sc in range(S // 512):
                        ps = ps_proj.tile([128, 512], F32, tag="proj")
                        for k in range(KD):
                            nc.tensor.matmul(
                                ps[:],
                                w_sb[:, k, ec * 128:(ec + 1) * 128],
                                xT_sb[:, k, sc * 512:(sc + 1) * 512],
                                start=(k == 0), stop=(k == KD - 1))
                        nc.vector.tensor_scalar(
                            out_sb[:, ec, sc * 512:(sc + 1) * 512], ps[:],
                            b_sb[:, ec:ec + 1], None, ADD)

            # V: [s-tile(128), e(256)] = xT.T @ wvT, scattered into V_aug
            for st in range(N_ST):
                ps = ps_proj.tile([128, 512], F32, tag="proj")
                for k in range(KD):
                    nc.tensor.matmul(
                        ps[:, 0:E],
                        xT_sb[:, k, st * 128:(st + 1) * 128],
                        wv_sb[:, k, :],
                        start=(k == 0), stop=(k == KD - 1))
                for h in range(4):
                    pr, odd = h // 2, h % 2
                    dst = (Vo_sb[pr][:, st, 64:128] if odd
                           else Ve_sb[pr][:, st, 0:64])
                    nc.vector.tensor_tensor(
                        dst, ps[:, h * 64:(h + 1) * 64],
                        bvb_sb[:, h * 64:(h + 1) * 64], ADD)

            # ---------------- phase 2+3: attention + out-proj ----------------
            for qc in range(N_QC):
                qsl = slice(qc * QC, (qc + 1) * QC)
                for pr in range(2):
                    pexp_e = wk.tile([128, N_KT, QC], F16, tag="pexp")
                    pexp_o = wk.tile([128, N_KT, QC], F16, tag="pexp")
                    for kt in range(N_KT):
                        ksl = slice(kt * 128, (kt + 1) * 128)
                        pse = ps_s.tile([128, QC], F32, tag="s")
                        pso = ps_s.tile([128, QC], F32, tag="s")
                        nc.tensor.matmul(pse[:], KT_sb[0:64, pr, ksl],
                                         QT_sb[0:64, pr, qsl])
                        nc.tensor.matmul(pso[:], KT_sb[64:128, pr, ksl],
                                         QT_sb[64:128, pr, qsl])
                        nc.scalar.activation(pexp_e[:, kt, :], pse[:],
                                             EXP, scale=0.125)
                        nc.scalar.activation(pexp_o[:, kt, :], pso[:],
                                             EXP, scale=0.125)

                    for odd, pexp in ((0, pexp_e), (1, pexp_o)):
                        v_sb = Vo_sb[pr] if odd else Ve_sb[pr]
                        psav = ps_av.tile([128, QC], F32, tag="av")
                        for kt in range(N_KT):
                            nc.tensor.matmul(
                                psav[:], v_sb[:, kt, :], pexp[:, kt, :],
                                start=(kt == 0), stop=(kt == N_KT - 1))
                        # normalization: sums at partition 64 (even) / 0 (odd)
                        sp = 0 if odd else 64      # sums partition
                        op = 64 if odd else 0      # hd base out partition
                        rec = wk.tile([128, QC], F32, tag="rec")
                        recr = wk.tile([128, QC], F16, tag="recr")
                        nc.vector.reciprocal(rec[sp:sp + 1, :],
                                             psav[sp:sp + 1, :])
                        nc.vector.tensor_copy(recr[sp:sp + 1, :],
                                              rec[sp:sp + 1, :])
                        psbc = ps_av.tile([128, QC], F32, tag="av")
                        nc.tensor.matmul(psbc[:], ones_sb[sp:sp + 1, :],
                                         recr[sp:sp + 1, :])
                        bcs = wk.tile([128, QC], F32, tag="bcs")
                        nc.vector.tensor_copy(bcs[op:op + 64, :],
                                              psbc[op:op + 64, :])
                        nc.vector.tensor_tensor(
                            OT_sb[op:op + 64, pr, qsl],
                            psav[op:op + 64, :] if odd else psav[0:64, :],
                            bcs[op:op + 64, :], MULT)

                # out-proj for the s-tiles covered by this q-chunk
                for sti in range(QC // 128):
                    st = qc * (QC // 128) + sti
                    ssl = slice(st * 128, (st + 1) * 128)
                    for nch in range(2):
                        psy = ps_proj.tile([128, 512], F32, tag="proj")
                        for cc in range(2):
                            nc.tensor.matmul(
                                psy[:], OT_sb[:, cc, ssl],
                                woT_sb[:, cc, nch * 512:(nch + 1) * 512],
                                start=(cc == 0), stop=(cc == 1))
                        y_sb = wk.tile([128, 512], F32, tag="y")
                        nc.vector.tensor_copy(y_sb[:], psy[:])
                        nc.sync.dma_start(
                            y[ssl, nch * 512:(nch + 1) * 512], y_sb[:])

    nc.compile()
    return nc


_NC_CACHE = None
last_in_maps = None


def kernel(x, Wq, bq, Wk, bk, Wv, bv, Wo, bo):
    global _NC_CACHE, last_in_maps
    x = np.asarray(x, dtype=np.float32)
    Wq, bq = np.asarray(Wq, np.float32), np.asarray(bq, np.float32)
    Wk, bk = np.asarray(Wk, np.float32), np.asarray(bk, np.float32)
    Wv, bv = np.asarray(Wv, np.float32), np.asarray(bv, np.float32)
    Wo, bo = np.asarray(Wo, np.float32), np.asarray(bo, np.float32)

    if _NC_CACHE is None:
        _NC_CACHE = build()
    nc = _NC_CACHE

    vcon = np.zeros((128, N_KT, 64), np.float16)
    vcon[:, :, 0] = 1.0
    ones128 = np.ones((1, 128), np.float16)

    in_maps = []
    for c in range(8):
        b, f0 = c // 4, (c % 4) * E
        fs = slice(f0, f0 + E)
        in_maps.append(dict(
            xT=np.ascontiguousarray(x[b].T).astype(np.float16),
            wqT=np.ascontiguousarray(Wq[fs, :].T).astype(np.float16),
            wkT=np.ascontiguousarray(Wk[fs, :].T).astype(np.float16),
            wvT=np.ascontiguousarray(Wv[fs, :].T).astype(np.float16),
            woT=np.ascontiguousarray(Wo[:, fs].T).astype(np.float16),
            bq2=np.ascontiguousarray(bq[fs].reshape(2, 128).T),
            bk2=np.ascontiguousarray(bk[fs].reshape(2, 128).T),
            bvb=np.ascontiguousarray(np.broadcast_to(bv[fs], (128, E))),
            vcon=vcon,
            ones128=ones128,
        ))

    last_in_maps = in_maps
    res = bass_utils.run_bass_kernel_spmd(nc, in_maps, core_ids=list(range(8)))

    out = np.zeros((B, S, D), np.float32)
    for c in range(8):
        out[c // 4] += res.results[c]["y"]
    out += bo
    return out


# revision 9
# speedup vs baseline: 1.8857x; 1.1098x over previous
"""Multi-head attention (B=2, S=2048, D=1024, H=16) on 8 TRN2 NeuronCores.

Sharding: core c handles batch b = c//4 and head group hg = c%4 (4 heads,
256 features f0 = hg*256). Each core computes Q/K/V projections for its
feature slice, attention for its 4 heads, and a partial output projection
y_partial = attnout @ Wo[:, f0:f0+256].T. Host sums the 4 partials per batch
and adds bo.

On-device layout strategy (everything contraction-dim-on-partitions):
 - host pre-transposes x -> xT [D, S] and weight slices -> [D, 256] so no
   on-device transposes are needed.
 - Q, K are produced transposed: QT/KT [e, s] (e = 256 local features).
 - scores are produced transposed per head: ST [k, q] = K_h @ Q_h.T, packed
   two heads at a time in the PE array (row groups 0-63 / 64-127, K=64 each).
 - softmax: exp(0.125 * ST) with no max subtraction (scores are ~N(0,1) by
   construction so exp is safe); row sums come from an extra ones column in
   the AV matmul; normalization multiplies by 1/sum broadcast across
   partitions via a K=1 ones-matmul.
 - AV: OT_h [hd, q] = V_aug_h.T @ Pexp (contraction over k, full K=128).
   Even heads: V at cols 0:64, ones at col 64 (sums at out partition 64);
   odd heads: ones at col 0, V at cols 64:128 (hd lands on partitions
   64:128). A normalized even/odd pair forms a full 128-partition
   attnout.T chunk.
 - out-proj: y[s-tile] = sum_cc OT[:, cc, s-tile].T @ woT[cc] (K=128 x2).

All matmuls run in float16 (1 cycle/row at 2.4GHz warm, FWL fast weight
load); accumulation is fp32 in PSUM, elementwise work is fp32 on DVE/ACT.
"""
import numpy as np

import concourse.bass as bass
import concourse.mybir as mybir
import concourse.tile as tile
from concourse import bacc
from concourse import bass_utils

F32 = mybir.dt.float32
F16 = mybir.dt.float16
EXP = mybir.ActivationFunctionType.Exp
ADD = mybir.AluOpType.add
MULT = mybir.AluOpType.mult

B, S, D, H = 2, 2048, 1024, 16
HD = D // H          # 64
E = 256              # local features per core (4 heads)
QC = 512             # q-chunk size for the attention phase
N_QC = S // QC       # 4
N_KT = S // 128      # 16 k-tiles
N_ST = S // 128      # 16 s-tiles
KD = D // 128        # 8 contraction tiles for the projections


def build():
    nc = bacc.Bacc("TRN2", target_bir_lowering=False, debug=False, num_devices=8)

    xT = nc.dram_tensor("xT", [D, S], F16, kind="ExternalInput").ap()
    wqT = nc.dram_tensor("wqT", [D, E], F16, kind="ExternalInput").ap()
    wkT = nc.dram_tensor("wkT", [D, E], F16, kind="ExternalInput").ap()
    wvT = nc.dram_tensor("wvT", [D, E], F16, kind="ExternalInput").ap()
    woT = nc.dram_tensor("woT", [E, D], F16, kind="ExternalInput").ap()
    bq2 = nc.dram_tensor("bq2", [128, 2], F32, kind="ExternalInput").ap()
    bk2 = nc.dram_tensor("bk2", [128, 2], F32, kind="ExternalInput").ap()
    bvb = nc.dram_tensor("bvb", [128, E], F32, kind="ExternalInput").ap()
    # [:, :, 0] = 1.0, rest 0 -- supplies the ones/zeros columns of V_aug
    vcon = nc.dram_tensor("vcon", [128, N_KT, 64], F16, kind="ExternalInput").ap()
    ones128 = nc.dram_tensor("ones128", [1, 128], F16, kind="ExternalInput").ap()

    y = nc.dram_tensor("y", [S, D], F32, kind="ExternalOutput").ap()

    with tile.TileContext(nc) as tc:
        with (
            tc.tile_pool(name="pool", bufs=1) as pp,
            tc.tile_pool(name="work", bufs=4) as wk,
            tc.tile_pool(name="ps_proj", bufs=2, space="PSUM") as ps_proj,
            tc.tile_pool(name="ps_s", bufs=3, space="PSUM") as ps_s,
            tc.tile_pool(name="ps_av", bufs=3, space="PSUM") as ps_av,
        ):
            # ---------------- persistent tiles ----------------
            woT_sb = pp.tile([128, 2, D], F16)
            bvb_sb = pp.tile([128, E], F32)
            vcon_sb = pp.tile([128, N_KT, 64], F16)
            ones_sb = pp.tile([128, 128], F16)     # partitions 0 and 64 used
            bq_sb = pp.tile([128, 2], F32)
            bk_sb = pp.tile([128, 2], F32)
            QT_sb = pp.tile([128, 2, S], F16)
            KT_sb = pp.tile([128, 2, S], F16)
            OT_sb = pp.tile([128, 2, S], F16)
            xT_sb = pp.tile([128, KD, S], F16)
            wq_sb = pp.tile([128, KD, E], F16)
            wk_sb = pp.tile([128, KD, E], F16)
            wv_sb = pp.tile([128, KD, E], F16)
            # V_aug per pair (128 cols each so the matmul dst is a full
            # 128-partition AP):
            #   even head: [*, kt, 0:64]=V, col 64=1, cols 65:128=0
            #   odd head:  col 0=1, cols 1:64=0, [*, kt, 64:128]=V
            Ve_sb = [pp.tile([128, N_KT, 128], F16, name=f"ve{p}", tag=f"ve{p}")
                     for p in range(2)]
            Vo_sb = [pp.tile([128, N_KT, 128], F16, name=f"vo{p}", tag=f"vo{p}")
                     for p in range(2)]

            for p in range(2):
                nc.sync.dma_start(woT_sb[:, p, :], woT[p * 128:(p + 1) * 128, :])
            nc.sync.dma_start(bvb_sb[:], bvb)
            nc.sync.dma_start(vcon_sb[:], vcon)
            nc.sync.dma_start(ones_sb[0:1, :], ones128)
            nc.sync.dma_start(ones_sb[64:65, :], ones128)
            nc.sync.dma_start(bq_sb[:], bq2)
            nc.sync.dma_start(bk_sb[:], bk2)
            for k in range(KD):
                nc.sync.dma_start(wq_sb[:, k, :], wqT[k * 128:(k + 1) * 128, :])
                nc.sync.dma_start(wk_sb[:, k, :], wkT[k * 128:(k + 1) * 128, :])
                nc.sync.dma_start(wv_sb[:, k, :], wvT[k * 128:(k + 1) * 128, :])
            for k in range(KD):
                nc.sync.dma_start(xT_sb[:, k, :], xT[k * 128:(k + 1) * 128, :])
            for pr in range(2):
                nc.sync.dma_start(Ve_sb[pr][:, :, 64:128], vcon_sb[:, :, :])
                nc.sync.dma_start(Vo_sb[pr][:, :, 0:64], vcon_sb[:, :, :])

            # ---------------- phase 1: projections ----------------
            # QT / KT: [e-chunk(128), s] = W.T @ xT
            for w_sb, b_sb, out_sb in ((wq_sb, bq_sb, QT_sb),
                                       (wk_sb, bk_sb, KT_sb)):
                for ec in range(2):
                    for sc in range(S // 512):
                        ps = ps_proj.tile([128, 512], F32, tag="proj")
                        for k in range(KD):
                            nc.tensor.matmul(
                                ps[:],
                                w_sb[:, k, ec * 128:(ec + 1) * 128],
                                xT_sb[:, k, sc * 512:(sc + 1) * 512],
                                start=(k == 0), stop=(k == KD - 1))
                        nc.vector.tensor_scalar(
                            out_sb[:, ec, sc * 512:(sc + 1) * 512], ps[:],
                            b_sb[:, ec:ec + 1], None, ADD)

            # V: [s-tile(128), e(256)] = xT.T @ wvT, scattered into V_aug
            for st in range(N_ST):
                ps = ps_proj.tile([128, 512], F32, tag="proj")
                for k in range(KD):
                    nc.tensor.matmul(
                        ps[:, 0:E],
                        xT_sb[:, k, st * 128:(st + 1) * 128],
                        wv_sb[:, k, :],
                        start=(k == 0), stop=(k == KD - 1))
                for h in range(4):
                    pr, odd = h // 2, h % 2
                    dst = (Vo_sb[pr][:, st, 64:128] if odd
                           else Ve_sb[pr][:, st, 0:64])
                    nc.vector.tensor_tensor(
                        dst, ps[:, h * 64:(h + 1) * 64],
                        bvb_sb[:, h * 64:(h + 1) * 64], ADD)

            # ---------------- phase 2+3: attention + out-proj ----------------
            for qc in range(N_QC):
                qsl = slice(qc * QC, (qc + 1) * QC)
                for pr in range(2):
                    pexp_e = wk.tile([128, N_KT, QC], F16, tag="pexp")
                    pexp_o = wk.tile([128, N_KT, QC], F16, tag="pexp")
                    for kt in range(N_KT):
                        ksl = slice(kt * 128, (kt + 1) * 128)
                        pse = ps_s.tile([128, QC], F32, tag="s")
                        pso = ps_s.tile([128, QC], F32, tag="s")
                        nc.tensor.matmul(pse[:], KT_sb[0:64, pr, ksl],
                                         QT_sb[0:64, pr, qsl])
                        nc.tensor.matmul(pso[:], KT_sb[64:128, pr, ksl],
                                         QT_sb[64:128, pr, qsl])
                        nc.scalar.activation(pexp_e[:, kt, :], pse[:],
                                             EXP, scale=0.125)
                        nc.scalar.activation(pexp_o[:, kt, :], pso[:],
                                             EXP, scale=0.125)

                    for odd, pexp in ((0, pexp_e), (1, pexp_o)):
                        v_sb = Vo_sb[pr] if odd else Ve_sb[pr]
                        psav = ps_av.tile([128, QC], F32, tag="av")
                        for kt in range(N_KT):
                            nc.tensor.matmul(
                                psav[:], v_sb[:, kt, :], pexp[:, kt, :],
                                start=(kt == 0), stop=(kt == N_KT - 1))
                        # normalization: sums at partition 64 (even) / 0 (odd);
                        # broadcast raw sums via ones-matmul, then approx-recip
                        # on the broadcast rows (same partitions as hd rows)
                        sp = 0 if odd else 64      # sums partition
                        op = 64 if odd else 0      # hd base out partition
                        sumr = wk.tile([128, QC], F16, tag="sumr")
                        nc.vector.tensor_copy(sumr[sp:sp + 1, :],
                                              psav[sp:sp + 1, :])
                        psbc = ps_av.tile([128, QC], F32, tag="av")
                        nc.tensor.matmul(psbc[:], ones_sb[sp:sp + 1, :],
                                         sumr[sp:sp + 1, :])
                        rec = wk.tile([128, QC], F32, tag="rec")
                        if op == 0:
                            # custom-DVE approx recip is only correct at
                            # base partition 0
                            nc.vector.reciprocal_approx_fast(
                                rec[0:64, :], psbc[0:64, :])
                        else:
                            nc.vector.reciprocal(rec[op:op + 64, :],
                                                 psbc[op:op + 64, :])
                        nc.vector.tensor_tensor(
                            OT_sb[op:op + 64, pr, qsl],
                            psav[op:op + 64, :] if odd else psav[0:64, :],
                            rec[op:op + 64, :], MULT)

                # out-proj for the s-tiles covered by this q-chunk
                for sti in range(QC // 128):
                    st = qc * (QC // 128) + sti
                    ssl = slice(st * 128, (st + 1) * 128)
                    for nch in range(2):
                        psy = ps_proj.tile([128, 512], F32, tag="proj")
                        for cc in range(2):
                            nc.tensor.matmul(
                                psy[:], OT_sb[:, cc, ssl],
                                woT_sb[:, cc, nch * 512:(nch + 1) * 512],
                                start=(cc == 0), stop=(cc == 1))
                        y_sb = wk.tile([128, 512], F32, tag="y")
                        nc.vector.tensor_copy(y_sb[:], psy[:])
                        nc.sync.dma_start(
                            y[ssl, nch * 512:(nch + 1) * 512], y_sb[:])

    nc.compile()
    return nc


_NC_CACHE = None
last_in_maps = None


def kernel(x, Wq, bq, Wk, bk, Wv, bv, Wo, bo):
    global _NC_CACHE, last_in_maps
    x = np.asarray(x, dtype=np.float32)
    Wq, bq = np.asarray(Wq, np.float32), np.asarray(bq, np.float32)
    Wk, bk = np.asarray(Wk, np.float32), np.asarray(bk, np.float32)
    Wv, bv = np.asarray(Wv, np.float32), np.asarray(bv, np.float32)
    Wo, bo = np.asarray(Wo, np.float32), np.asarray(bo, np.float32)

    if _NC_CACHE is None:
        _NC_CACHE = build()
    nc = _NC_CACHE

    vcon = np.zeros((128, N_KT, 64), np.float16)
    vcon[:, :, 0] = 1.0
    ones128 = np.ones((1, 128), np.float16)

    in_maps = []
    for c in range(8):
        b, f0 = c // 4, (c % 4) * E
        fs = slice(f0, f0 + E)
        in_maps.append(dict(
            xT=np.ascontiguousarray(x[b].T).astype(np.float16),
            wqT=np.ascontiguousarray(Wq[fs, :].T).astype(np.float16),
            wkT=np.ascontiguousarray(Wk[fs, :].T).astype(np.float16),
            wvT=np.ascontiguousarray(Wv[fs, :].T).astype(np.float16),
            woT=np.ascontiguousarray(Wo[:, fs].T).astype(np.float16),
            bq2=np.ascontiguousarray(bq[fs].reshape(2, 128).T),
            bk2=np.ascontiguousarray(bk[fs].reshape(2, 128).T),
            bvb=np.ascontiguousarray(np.broadcast_to(bv[fs], (128, E))),
            vcon=vcon,
            ones128=ones128,
        ))

    last_in_maps = in_maps
    res = bass_utils.run_bass_kernel_spmd(nc, in_maps, core_ids=list(range(8)))

    out = np.zeros((B, S, D), np.float32)
    for c in range(8):
        out[c // 4] += res.results[c]["y"]
    out += bo
    return out


# revision 10
# speedup vs baseline: 2.1764x; 1.1542x over previous
"""Multi-head attention (B=2, S=2048, D=1024, H=16) on 8 TRN2 NeuronCores.

Sharding: core c handles batch b = c//4 and head group hg = c%4 (4 heads,
256 features f0 = hg*256). Each core computes Q/K/V projections for its
feature slice, attention for its 4 heads, and a partial output projection
y_partial = attnout @ Wo[:, f0:f0+256].T. Host sums the 4 partials per batch
and adds bo.

On-device layout strategy (everything contraction-dim-on-partitions):
 - host pre-transposes x -> xT [D, S] and weight slices -> [D, 256] so no
   on-device transposes are needed.
 - Q, K are produced transposed: QT/KT [e, s] (e = 256 local features).
 - scores are produced transposed per head: ST [k, q] = K_h @ Q_h.T, packed
   two heads at a time in the PE array (row groups 0-63 / 64-127, K=64 each).
 - softmax: exp(0.125 * ST) with no max subtraction (scores are ~N(0,1) by
   construction so exp is safe); row sums come from an extra ones column in
   the AV matmul; normalization multiplies by 1/sum broadcast across
   partitions via a K=1 ones-matmul.
 - AV: OT_h [hd, q] = V_aug_h.T @ Pexp (contraction over k, full K=128).
   Even heads: V at cols 0:64, ones at col 64 (sums at out partition 64);
   odd heads: ones at col 0, V at cols 64:128 (hd lands on partitions
   64:128). A normalized even/odd pair forms a full 128-partition
   attnout.T chunk.
 - out-proj: y[s-tile] = sum_cc OT[:, cc, s-tile].T @ woT[cc] (K=128 x2).

All matmuls run in float16 (1 cycle/row at 2.4GHz warm, FWL fast weight
load); accumulation is fp32 in PSUM, elementwise work is fp32 on DVE/ACT.
"""
import numpy as np

import concourse.bass as bass
import concourse.mybir as mybir
import concourse.tile as tile
from concourse import bacc
from concourse import bass_utils

F32 = mybir.dt.float32
F16 = mybir.dt.float16
EXP = mybir.ActivationFunctionType.Exp
ADD = mybir.AluOpType.add
MULT = mybir.AluOpType.mult

B, S, D, H = 2, 2048, 1024, 16
HD = D // H          # 64
E = 256              # local features per core (4 heads)
QC = 512             # q-chunk size for the attention phase
N_QC = S // QC       # 4
N_KT = S // 128      # 16 k-tiles
N_ST = S // 128      # 16 s-tiles
KD = D // 128        # 8 contraction tiles for the projections


def build():
    nc = bacc.Bacc("TRN2", target_bir_lowering=False, debug=False, num_devices=8)

    xT = nc.dram_tensor("xT", [D, S], F16, kind="ExternalInput").ap()
    wqT = nc.dram_tensor("wqT", [D, E], F16, kind="ExternalInput").ap()
    wkT = nc.dram_tensor("wkT", [D, E], F16, kind="ExternalInput").ap()
    wvT = nc.dram_tensor("wvT", [D, E], F16, kind="ExternalInput").ap()
    woT = nc.dram_tensor("woT", [E, D], F16, kind="ExternalInput").ap()
    bq2 = nc.dram_tensor("bq2", [128, 2], F32, kind="ExternalInput").ap()
    bk2 = nc.dram_tensor("bk2", [128, 2], F32, kind="ExternalInput").ap()
    bvb = nc.dram_tensor("bvb", [128, E], F32, kind="ExternalInput").ap()
    # [:, :, 0] = 1.0, rest 0 -- supplies the ones/zeros columns of V_aug
    vcon = nc.dram_tensor("vcon", [128, N_KT, 64], F16, kind="ExternalInput").ap()
    ones128 = nc.dram_tensor("ones128", [1, 128], F16, kind="ExternalInput").ap()

    y = nc.dram_tensor("y", [S, D], F32, kind="ExternalOutput").ap()

    with tile.TileContext(nc) as tc:
        with (
            tc.tile_pool(name="pool", bufs=1) as pp,
            tc.tile_pool(name="work", bufs=4) as wk,
            tc.tile_pool(name="ps_proj", bufs=2, space="PSUM") as ps_proj,
            tc.tile_pool(name="ps_s", bufs=3, space="PSUM") as ps_s,
            tc.tile_pool(name="ps_av", bufs=3, space="PSUM") as ps_av,
        ):
            # ---------------- persistent tiles ----------------
            woT_sb = pp.tile([128, 2, D], F16)
            bvb_sb = pp.tile([128, E], F32)
            vcon_sb = pp.tile([128, N_KT, 64], F16)
            ones_sb = pp.tile([128, 128], F16)     # partitions 0 and 64 used
            bq_sb = pp.tile([128, 2], F32)
            bk_sb = pp.tile([128, 2], F32)
            QT_sb = pp.tile([128, 2, S], F16)
            KT_sb = pp.tile([128, 2, S], F16)
            OT_sb = pp.tile([128, 2, S], F16)
            xT_sb = pp.tile([128, KD, S], F16)
            wq_sb = pp.tile([128, KD, E], F16)
            wk_sb = pp.tile([128, KD, E], F16)
            wv_sb = pp.tile([128, KD, E], F16)
            # V_aug per pair (128 cols each so the matmul dst is a full
            # 128-partition AP):
            #   even head: [*, kt, 0:64]=V, col 64=1, cols 65:128=0
            #   odd head:  col 0=1, cols 1:64=0, [*, kt, 64:128]=V
            Ve_sb = [pp.tile([128, N_KT, 128], F16, name=f"ve{p}", tag=f"ve{p}")
                     for p in range(2)]
            Vo_sb = [pp.tile([128, N_KT, 128], F16, name=f"vo{p}", tag=f"vo{p}")
                     for p in range(2)]

            for p in range(2):
                nc.sync.dma_start(woT_sb[:, p, :], woT[p * 128:(p + 1) * 128, :])
            nc.sync.dma_start(bvb_sb[:], bvb)
            nc.sync.dma_start(vcon_sb[:], vcon)
            nc.sync.dma_start(ones_sb[0:1, :], ones128)
            nc.sync.dma_start(ones_sb[64:65, :], ones128)
            nc.sync.dma_start(bq_sb[:], bq2)
            nc.sync.dma_start(bk_sb[:], bk2)
            for k in range(KD):
                nc.sync.dma_start(wq_sb[:, k, :], wqT[k * 128:(k + 1) * 128, :])
                nc.sync.dma_start(wk_sb[:, k, :], wkT[k * 128:(k + 1) * 128, :])
                nc.sync.dma_start(wv_sb[:, k, :], wvT[k * 128:(k + 1) * 128, :])
            # stream xT in 512-column blocks in the order the Q/K
            # projection consumes them
            for scb in range(S // 512):
                for k in range(KD):
                    nc.sync.dma_start(
                        xT_sb[:, k, scb * 512:(scb + 1) * 512],
                        xT[k * 128:(k + 1) * 128, scb * 512:(scb + 1) * 512])
            for pr in range(2):
                nc.sync.dma_start(Ve_sb[pr][:, :, 64:128], vcon_sb[:, :, :])
                nc.sync.dma_start(Vo_sb[pr][:, :, 0:64], vcon_sb[:, :, :])

            # ---------------- phase 1: projections ----------------
            # QT / KT: [e-chunk(128), s] = W.T @ xT
            for w_sb, b_sb, out_sb in ((wq_sb, bq_sb, QT_sb),
                                       (wk_sb, bk_sb, KT_sb)):
                for ec in range(2):
                    for sc in range(S // 512):
                        ps = ps_proj.tile([128, 512], F32, tag="proj")
                        for k in range(KD):
                            nc.tensor.matmul(
                                ps[:],
                                w_sb[:, k, ec * 128:(ec + 1) * 128],
                                xT_sb[:, k, sc * 512:(sc + 1) * 512],
                                start=(k == 0), stop=(k == KD - 1))
                        nc.vector.tensor_scalar(
                            out_sb[:, ec, sc * 512:(sc + 1) * 512], ps[:],
                            b_sb[:, ec:ec + 1], None, ADD)

            # V: [s-tile(128), e(256)] = xT.T @ wvT, scattered into V_aug
            for st in range(N_ST):
                ps = ps_proj.tile([128, 512], F32, tag="proj")
                for k in range(KD):
                    nc.tensor.matmul(
                        ps[:, 0:E],
                        xT_sb[:, k, st * 128:(st + 1) * 128],
                        wv_sb[:, k, :],
                        start=(k == 0), stop=(k == KD - 1))
                for h in range(4):
                    pr, odd = h // 2, h % 2
                    dst = (Vo_sb[pr][:, st, 64:128] if odd
                           else Ve_sb[pr][:, st, 0:64])
                    nc.vector.tensor_tensor(
                        dst, ps[:, h * 64:(h + 1) * 64],
                        bvb_sb[:, h * 64:(h + 1) * 64], ADD)

            # ---------------- phase 2+3: attention + out-proj ----------------
            for qc in range(N_QC):
                qsl = slice(qc * QC, (qc + 1) * QC)
                for pr in range(2):
                    pexp_e = wk.tile([128, N_KT, QC], F16, tag="pexp")
                    pexp_o = wk.tile([128, N_KT, QC], F16, tag="pexp")
                    for kt in range(N_KT):
                        ksl = slice(kt * 128, (kt + 1) * 128)
                        pse = ps_s.tile([128, QC], F32, tag="s")
                        pso = ps_s.tile([128, QC], F32, tag="s")
                        nc.tensor.matmul(pse[:], KT_sb[0:64, pr, ksl],
                                         QT_sb[0:64, pr, qsl])
                        nc.tensor.matmul(pso[:], KT_sb[64:128, pr, ksl],
                                         QT_sb[64:128, pr, qsl])
                        nc.scalar.activation(pexp_e[:, kt, :], pse[:],
                                             EXP, scale=0.125)
                        nc.scalar.activation(pexp_o[:, kt, :], pso[:],
                                             EXP, scale=0.125)

                    # odd head first: its slow reciprocal hides under the
                    # even head's AV matmul stream
                    for odd, pexp in ((1, pexp_o), (0, pexp_e)):
                        v_sb = Vo_sb[pr] if odd else Ve_sb[pr]
                        psav = ps_av.tile([128, QC], F32, tag="av")
                        for kt in range(N_KT):
                            nc.tensor.matmul(
                                psav[:], v_sb[:, kt, :], pexp[:, kt, :],
                                start=(kt == 0), stop=(kt == N_KT - 1))
                        # normalization: sums at partition 64 (even) / 0 (odd);
                        # broadcast raw sums via ones-matmul, then approx-recip
                        # on the broadcast rows (same partitions as hd rows)
                        sp = 0 if odd else 64      # sums partition
                        op = 64 if odd else 0      # hd base out partition
                        sumr = wk.tile([128, QC], F16, tag="sumr")
                        nc.vector.tensor_copy(sumr[sp:sp + 1, :],
                                              psav[sp:sp + 1, :])
                        psbc = ps_av.tile([128, QC], F32, tag="av")
                        nc.tensor.matmul(psbc[:], ones_sb[sp:sp + 1, :],
                                         sumr[sp:sp + 1, :])
                        rec = wk.tile([128, QC], F32, tag="rec")
                        if op == 0:
                            # custom-DVE approx recip is only correct at
                            # base partition 0
                            nc.vector.reciprocal_approx_fast(
                                rec[0:64, :], psbc[0:64, :])
                        else:
                            nc.vector.reciprocal(rec[op:op + 64, :],
                                                 psbc[op:op + 64, :])
                        nc.vector.tensor_tensor(
                            OT_sb[op:op + 64, pr, qsl],
                            psav[op:op + 64, :] if odd else psav[0:64, :],
                            rec[op:op + 64, :], MULT)

                # out-proj for the s-tiles covered by this q-chunk
                for sti in range(QC // 128):
                    st = qc * (QC // 128) + sti
                    ssl = slice(st * 128, (st + 1) * 128)
                    for nch in range(2):
                        psy = ps_proj.tile([128, 512], F32, tag="proj")
                        for cc in range(2):
                            nc.tensor.matmul(
                                psy[:], OT_sb[:, cc, ssl],
                                woT_sb[:, cc, nch * 512:(nch + 1) * 512],
                                start=(cc == 0), stop=(cc == 1))
                        y_sb = wk.tile([128, 512], F32, tag="y")
                        nc.vector.tensor_copy(y_sb[:], psy[:])
                        nc.sync.dma_start(
                            y[ssl, nch * 512:(nch + 1) * 512], y_sb[:])

    nc.compile()
    return nc


_NC_CACHE = None
last_in_maps = None


def kernel(x, Wq, bq, Wk, bk, Wv, bv, Wo, bo):
    global _NC_CACHE, last_in_maps
    x = np.asarray(x, dtype=np.float32)
    Wq, bq = np.asarray(Wq, np.float32), np.asarray(bq, np.float32)
    Wk, bk = np.asarray(Wk, np.float32), np.asarray(bk, np.float32)
    Wv, bv = np.asarray(Wv, np.float32), np.asarray(bv, np.float32)
    Wo, bo = np.asarray(Wo, np.float32), np.asarray(bo, np.float32)

    if _NC_CACHE is None:
        _NC_CACHE = build()
    nc = _NC_CACHE

    vcon = np.zeros((128, N_KT, 64), np.float16)
    vcon[:, :, 0] = 1.0
    ones128 = np.ones((1, 128), np.float16)

    in_maps = []
    for c in range(8):
        b, f0 = c // 4, (c % 4) * E
        fs = slice(f0, f0 + E)
        in_maps.append(dict(
            xT=np.ascontiguousarray(x[b].T).astype(np.float16),
            wqT=np.ascontiguousarray(Wq[fs, :].T).astype(np.float16),
            wkT=np.ascontiguousarray(Wk[fs, :].T).astype(np.float16),
            wvT=np.ascontiguousarray(Wv[fs, :].T).astype(np.float16),
            woT=np.ascontiguousarray(Wo[:, fs].T).astype(np.float16),
            bq2=np.ascontiguousarray(bq[fs].reshape(2, 128).T),
            bk2=np.ascontiguousarray(bk[fs].reshape(2, 128).T),
            bvb=np.ascontiguousarray(np.broadcast_to(bv[fs], (128, E))),
            vcon=vcon,
            ones128=ones128,
        ))

    last_in_maps = in_maps
    res = bass_utils.run_bass_kernel_spmd(nc, in_maps, core_ids=list(range(8)))

    out = np.zeros((B, S, D), np.float32)
    for c in range(8):
        out[c // 4] += res.results[c]["y"]
    out += bo
    return out


# revision 16
# speedup vs baseline: 2.1826x; 1.0028x over previous
"""Multi-head attention (B=2, S=2048, D=1024, H=16) on 8 TRN2 NeuronCores.

Sharding: core c handles batch b = c//4 and head group hg = c%4 (4 heads,
256 features f0 = hg*256). Each core computes Q/K/V projections for its
feature slice, attention for its 4 heads, and a partial output projection
y_partial = attnout @ Wo[:, f0:f0+256].T. Host sums the 4 partials per batch
and adds bo.

On-device layout strategy (everything contraction-dim-on-partitions):
 - host pre-transposes x -> xT [D, S] and weight slices -> [D, 256] so no
   on-device transposes are needed.
 - Q, K are produced transposed: QT/KT [e, s] (e = 256 local features).
 - scores are produced transposed per head: ST [k, q] = K_h @ Q_h.T, packed
   two heads at a time in the PE array (row groups 0-63 / 64-127, K=64 each).
 - softmax: exp(0.125 * ST) with no max subtraction (scores are ~N(0,1) by
   construction so exp is safe); row sums come from an extra ones column in
   the AV matmul; normalization multiplies by 1/sum broadcast across
   partitions via a K=1 ones-matmul.
 - AV: OT_h [hd, q] = V_aug_h.T @ Pexp (contraction over k, full K=128).
   Even heads: V at cols 0:64, ones at col 64 (sums at out partition 64);
   odd heads: ones at col 0, V at cols 64:128 (hd lands on partitions
   64:128). A normalized even/odd pair forms a full 128-partition
   attnout.T chunk.
 - out-proj: y[s-tile] = sum_cc OT[:, cc, s-tile].T @ woT[cc] (K=128 x2).

All matmuls run in float16 (1 cycle/row at 2.4GHz warm, FWL fast weight
load); accumulation is fp32 in PSUM, elementwise work is fp32 on DVE/ACT.
"""
import numpy as np

import concourse.bass as bass
import concourse.mybir as mybir
import concourse.tile as tile
from concourse import bacc
from concourse import bass_utils

F32 = mybir.dt.float32
F16 = mybir.dt.float16
EXP = mybir.ActivationFunctionType.Exp
ADD = mybir.AluOpType.add
MULT = mybir.AluOpType.mult

B, S, D, H = 2, 2048, 1024, 16
HD = D // H          # 64
E = 256              # local features per core (4 heads)
QC = 512             # q-chunk size for the attention phase
N_QC = S // QC       # 4
N_KT = S // 128      # 16 k-tiles
N_ST = S // 128      # 16 s-tiles
KD = D // 128        # 8 contraction tiles for the projections


def build():
    nc = bacc.Bacc("TRN2", target_bir_lowering=False, debug=False, num_devices=8)

    xT = nc.dram_tensor("xT", [D, S], F16, kind="ExternalInput").ap()
    wqT = nc.dram_tensor("wqT", [D, E], F16, kind="ExternalInput").ap()
    wkT = nc.dram_tensor("wkT", [D, E], F16, kind="ExternalInput").ap()
    wvT = nc.dram_tensor("wvT", [D, E], F16, kind="ExternalInput").ap()
    woT = nc.dram_tensor("woT", [E, D], F16, kind="ExternalInput").ap()
    bq2 = nc.dram_tensor("bq2", [128, 2], F32, kind="ExternalInput").ap()
    bk2 = nc.dram_tensor("bk2", [128, 2], F32, kind="ExternalInput").ap()
    bvb = nc.dram_tensor("bvb", [128, E], F32, kind="ExternalInput").ap()
    # [:, :, 0] = 1.0, rest 0 -- supplies the ones/zeros columns of V_aug
    vcon = nc.dram_tensor("vcon", [128, N_KT, 64], F16, kind="ExternalInput").ap()
    ones128 = nc.dram_tensor("ones128", [1, 128], F16, kind="ExternalInput").ap()

    y = nc.dram_tensor("y", [S, D], F32, kind="ExternalOutput").ap()

    with tile.TileContext(nc) as tc:
        with (
            tc.tile_pool(name="pool", bufs=1) as pp,
            tc.tile_pool(name="work", bufs=4) as wk,
            tc.tile_pool(name="ps_proj", bufs=2, space="PSUM") as ps_proj,
            tc.tile_pool(name="ps_s", bufs=3, space="PSUM") as ps_s,
            tc.tile_pool(name="ps_av", bufs=3, space="PSUM") as ps_av,
        ):
            # ---------------- persistent tiles ----------------
            woT_sb = pp.tile([128, 2, D], F16)
            bvb_sb = pp.tile([128, E], F32)
            vcon_sb = pp.tile([128, N_KT, 64], F16)
            ones_sb = pp.tile([128, 128], F16)     # partitions 0 and 64 used
            bq_sb = pp.tile([128, 2], F32)
            bk_sb = pp.tile([128, 2], F32)
            QT_sb = pp.tile([128, 2, S], F16)
            KT_sb = pp.tile([128, 2, S], F16)
            OT_sb = pp.tile([128, 2, S], F16)
            xT_sb = pp.tile([128, KD, S], F16)
            wq_sb = pp.tile([128, KD, E], F16)
            wk_sb = pp.tile([128, KD, E], F16)
            wv_sb = pp.tile([128, KD, E], F16)
            # V_aug per pair (128 cols each so the matmul dst is a full
            # 128-partition AP):
            #   even head: [*, kt, 0:64]=V, col 64=1, cols 65:128=0
            #   odd head:  col 0=1, cols 1:64=0, [*, kt, 64:128]=V
            Ve_sb = [pp.tile([128, N_KT, 128], F16, name=f"ve{p}", tag=f"ve{p}")
                     for p in range(2)]
            Vo_sb = [pp.tile([128, N_KT, 128], F16, name=f"vo{p}", tag=f"vo{p}")
                     for p in range(2)]

            # Spread DMA issue across engine queues: the issuing engine pays
            # ~650ns per dma_start, so a single queue serializes the input
            # loads. Weights on sync (wq first), xT on gpsimd, constants on
            # scalar.
            for k in range(KD):
                nc.sync.dma_start(wq_sb[:, k, :], wqT[k * 128:(k + 1) * 128, :])
            for k in range(KD):
                nc.sync.dma_start(wk_sb[:, k, :], wkT[k * 128:(k + 1) * 128, :])
            for k in range(KD):
                nc.sync.dma_start(wv_sb[:, k, :], wvT[k * 128:(k + 1) * 128, :])
            # stream xT in 512-column blocks in the order the Q/K
            # projection consumes them
            for scb in range(S // 512):
                for k in range(KD):
                    nc.gpsimd.dma_start(
                        xT_sb[:, k, scb * 512:(scb + 1) * 512],
                        xT[k * 128:(k + 1) * 128, scb * 512:(scb + 1) * 512])
            for p in range(2):
                nc.scalar.dma_start(woT_sb[:, p, :], woT[p * 128:(p + 1) * 128, :])
            nc.scalar.dma_start(bvb_sb[:], bvb)
            nc.scalar.dma_start(vcon_sb[:], vcon)
            nc.scalar.dma_start(ones_sb[0:1, :], ones128)
            nc.scalar.dma_start(ones_sb[64:65, :], ones128)
            nc.scalar.dma_start(bq_sb[:], bq2)
            nc.scalar.dma_start(bk_sb[:], bk2)
            for pr in range(2):
                nc.scalar.dma_start(Ve_sb[pr][:, :, 64:128], vcon_sb[:, :, :])
                nc.scalar.dma_start(Vo_sb[pr][:, :, 0:64], vcon_sb[:, :, :])

            # ---------------- phase 1: projections ----------------
            # QT / KT: [e-chunk(128), s] = W.T @ xT
            for w_sb, b_sb, out_sb in ((wq_sb, bq_sb, QT_sb),
                                       (wk_sb, bk_sb, KT_sb)):
                for ec in range(2):
                    for sc in range(S // 512):
                        ps = ps_proj.tile([128, 512], F32, tag="proj")
                        for k in range(KD):
                            nc.tensor.matmul(
                                ps[:],
                                w_sb[:, k, ec * 128:(ec + 1) * 128],
                                xT_sb[:, k, sc * 512:(sc + 1) * 512],
                                start=(k == 0), stop=(k == KD - 1))
                        nc.vector.tensor_scalar(
                            out_sb[:, ec, sc * 512:(sc + 1) * 512], ps[:],
                            b_sb[:, ec:ec + 1], None, ADD)

            # V: [s-tile(128), e(256)] = xT.T @ wvT, scattered into V_aug
            for st in range(N_ST):
                ps = ps_proj.tile([128, 512], F32, tag="proj")
                for k in range(KD):
                    nc.tensor.matmul(
                        ps[:, 0:E],
                        xT_sb[:, k, st * 128:(st + 1) * 128],
                        wv_sb[:, k, :],
                        start=(k == 0), stop=(k == KD - 1))
                for h in range(4):
                    pr, odd = h // 2, h % 2
                    dst = (Vo_sb[pr][:, st, 64:128] if odd
                           else Ve_sb[pr][:, st, 0:64])
                    nc.vector.tensor_tensor(
                        dst, ps[:, h * 64:(h + 1) * 64],
                        bvb_sb[:, h * 64:(h + 1) * 64], ADD)

            # ---------------- phase 2+3: attention + out-proj ----------------
            for qc in range(N_QC):
                qsl = slice(qc * QC, (qc + 1) * QC)
                for pr in range(2):
                    pexp_e = wk.tile([128, N_KT, QC], F16, tag="pexp")
                    pexp_o = wk.tile([128, N_KT, QC], F16, tag="pexp")
                    for kt in range(N_KT):
                        ksl = slice(kt * 128, (kt + 1) * 128)
                        pse = ps_s.tile([128, QC], F32, tag="s")
                        pso = ps_s.tile([128, QC], F32, tag="s")
                        nc.tensor.matmul(pse[:], KT_sb[0:64, pr, ksl],
                                         QT_sb[0:64, pr, qsl])
                        nc.tensor.matmul(pso[:], KT_sb[64:128, pr, ksl],
                                         QT_sb[64:128, pr, qsl])
                        nc.scalar.activation(pexp_e[:, kt, :], pse[:],
                                             EXP, scale=0.125)
                        nc.scalar.activation(pexp_o[:, kt, :], pso[:],
                                             EXP, scale=0.125)

                    # odd head first: its slow reciprocal hides under the
                    # even head's AV matmul stream
                    for odd, pexp in ((1, pexp_o), (0, pexp_e)):
                        v_sb = Vo_sb[pr] if odd else Ve_sb[pr]
                        psav = ps_av.tile([128, QC], F32, tag="av")
                        for kt in range(N_KT):
                            nc.tensor.matmul(
                                psav[:], v_sb[:, kt, :], pexp[:, kt, :],
                                start=(kt == 0), stop=(kt == N_KT - 1))
                        # normalization: sums at partition 64 (even) / 0 (odd);
                        # broadcast raw sums via ones-matmul, then approx-recip
                        # on the broadcast rows (same partitions as hd rows)
                        sp = 0 if odd else 64      # sums partition
                        op = 64 if odd else 0      # hd base out partition
                        sumr = wk.tile([128, QC], F16, tag="sumr")
                        nc.vector.tensor_copy(sumr[sp:sp + 1, :],
                                              psav[sp:sp + 1, :])
                        psbc = ps_av.tile([128, QC], F32, tag="av")
                        nc.tensor.matmul(psbc[:], ones_sb[sp:sp + 1, :],
                                         sumr[sp:sp + 1, :])
                        rec = wk.tile([128, QC], F32, tag="rec")
                        if op == 0:
                            # custom-DVE approx recip is only correct at
                            # base partition 0
                            nc.vector.reciprocal_approx_fast(
                                rec[0:64, :], psbc[0:64, :])
                        else:
                            nc.vector.reciprocal(rec[op:op + 64, :],
                                                 psbc[op:op + 64, :])
                        nc.vector.tensor_tensor(
                            OT_sb[op:op + 64, pr, qsl],
                            psav[op:op + 64, :] if odd else psav[0:64, :],
                            rec[op:op + 64, :], MULT)

                # out-proj for the s-tiles covered by this q-chunk
                for sti in range(QC // 128):
                    st = qc * (QC // 128) + sti
                    ssl = slice(st * 128, (st + 1) * 128)
                    for nch in range(2):
                        psy = ps_proj.tile([128, 512], F32, tag="proj")
                        for cc in range(2):
                            nc.tensor.matmul(
                                psy[:], OT_sb[:, cc, ssl],
                                woT_sb[:, cc, nch * 512:(nch + 1) * 512],
                                start=(cc == 0), stop=(cc == 1))
                        y_sb = wk.tile([128, 512], F32, tag="y")
                        nc.vector.tensor_copy(y_sb[:], psy[:])
                        nc.gpsimd.dma_start(
                            y[ssl, nch * 512:(nch + 1) * 512], y_sb[:])

    nc.compile()
    return nc


_NC_CACHE = None
last_in_maps = None


def kernel(x, Wq, bq, Wk, bk, Wv, bv, Wo, bo):
    global _NC_CACHE, last_in_maps
    x = np.asarray(x, dtype=np.float32)
    Wq, bq = np.asarray(Wq, np.float32), np.asarray(bq, np.float32)
    Wk, bk = np.asarray(Wk, np.float32), np.asarray(bk, np.float32)
    Wv, bv = np.asarray(Wv, np.float32), np.asarray(bv, np.float32)
    Wo, bo = np.asarray(Wo, np.float32), np.asarray(bo, np.float32)

    if _NC_CACHE is None:
        _NC_CACHE = build()
    nc = _NC_CACHE

    vcon = np.zeros((128, N_KT, 64), np.float16)
    vcon[:, :, 0] = 1.0
    ones128 = np.ones((1, 128), np.float16)

    in_maps = []
    for c in range(8):
        b, f0 = c // 4, (c % 4) * E
        fs = slice(f0, f0 + E)
        in_maps.append(dict(
            xT=np.ascontiguousarray(x[b].T).astype(np.float16),
            wqT=np.ascontiguousarray(Wq[fs, :].T).astype(np.float16),
            wkT=np.ascontiguousarray(Wk[fs, :].T).astype(np.float16),
            wvT=np.ascontiguousarray(Wv[fs, :].T).astype(np.float16),
            woT=np.ascontiguousarray(Wo[:, fs].T).astype(np.float16),
            bq2=np.ascontiguousarray(bq[fs].reshape(2, 128).T),
            bk2=np.ascontiguousarray(bk[fs].reshape(2, 128).T),
            bvb=np.ascontiguousarray(np.broadcast_to(bv[fs], (128, E))),
            vcon=vcon,
            ones128=ones128,
        ))

    last_in_maps = in_maps
    res = bass_utils.run_bass_kernel_spmd(nc, in_maps, core_ids=list(range(8)))

    out = np.zeros((B, S, D), np.float32)
    for c in range(8):
        out[c // 4] += res.results[c]["y"]
    out += bo
    return out
